# revision 40
# baseline (speedup 1.0000x reference)
"""Causal GQA self-attention (B=2, S=2048, H=16, HKV=4, D=128) on 8 trn2 cores.

Sharding: tensor-parallel over kv-heads (4-way, Megatron-style) x data-parallel
over batch (2-way). Core (b, g) handles batch b, q-heads 4g..4g+3, kv-head g.
Each core returns a partial output [DIM, S] (transposed, bf16); the host
transposes and sums the 4 TP partials per batch.

Device design notes (engine balance, from the TRN2 cost model):
  - All activations feature-major ([feat, seq]); everything through PE in bf16.
  - Projection runs m-outer (2 PSUM banks rotating) with per-m weight DMAs
    interleaved with kt-group slices of the first x chunk, so the first
    matmul issues ~1us in and the stream never starves.
  - rmsnorm smalls are software-pipelined 1-3 m-groups behind the projection
    matmuls; rope runs in bf16 on DVE with the half-swap done by two
    SBUF->SBUF DMAs issued from the ACT queue.
  - Attention runs per chunk as head-group passes: per k-tile, score matmuls,
    one grouped exp on ACT, causal mask via affine_select on GPSIMD (diagonal
    tiles only, which run FIRST via the reversed-t order so the pass-end
    chain is short), softmax denominator accumulated on DVE in bf16, and AV
    matmuls skewed one step behind the scores so PE never waits on exp.
  - Chunk 0's attention (1-head passes, 3 spare PSUM banks) is interleaved
    directly into the projection matmul stream of chunks 2-3; chunks 1-3 run
    in phase A as 2-head passes with the output projection (wo stationary,
    output transposed) emitted as PE filler between attention steps.
  - Softmax needs no max-subtraction: rms-normed q,k bound
    |scores|/sqrt(D) <= sqrt(D) ~ 11.3.
"""

import contextlib
import itertools
import os
import sys

import numpy as np

for _p in ("/opt/trn_rl_repo", "/root/.axon_site/_ro/trn_rl_repo"):
    if os.path.isdir(_p) and _p not in sys.path:
        sys.path.insert(0, _p)

import concourse.bass as bass
import concourse.bacc as bacc
import concourse.tile as tile
from concourse import mybir
from concourse.bass_utils import run_bass_kernel_spmd
from concourse.masks import make_identity
import ml_dtypes

F32 = mybir.dt.float32
F32R = mybir.dt.float32r
BF16 = mybir.dt.bfloat16

B, S, H, HKV, D = 2, 2048, 16, 4, 128
DIM = H * D            # 2048
G = 4                  # TP degree (kv heads)
HQ = H // HKV          # q heads per core = 4
MQ = HQ * D            # 512 q features per core
KM = MQ + 2 * D        # 768 = q(512) + k(128) + v(128) projection rows per core
NKT = DIM // 128       # 16 k-tiles of the contraction dim
NCH = S // 512         # 4 seq chunks of 512
NST = S // 128         # 16 seq tiles of 128
SCALE = float(1.0 / np.sqrt(D))
EPS = float(np.finfo(np.float32).eps)

_CACHED = {}


def _build_program(loop=1):
    """loop > 1 wraps the whole computation in a device-side For_i so one
    dispatch runs the body `loop` times — used only by timing harnesses
    (wall(K) - wall(1) isolates device time from dispatch overhead)."""
    nc = bacc.Bacc("TRN2", target_bir_lowering=False)

    xt_d = nc.dram_tensor("xt", [DIM, S], BF16, kind="ExternalInput")
    wqkv_d = nc.dram_tensor("wqkv", [128, 6 * NKT * 128], BF16,
                            kind="ExternalInput")
    wot_d = nc.dram_tensor("wot", [128, HQ * NST * 128], BF16,
                           kind="ExternalInput")
    cosf_d = nc.dram_tensor("cosf", [128, S], BF16, kind="ExternalInput")
    sinf_d = nc.dram_tensor("sinf", [128, S], BF16, kind="ExternalInput")
    qg_d = nc.dram_tensor("qg", [1, HQ], F32, kind="ExternalInput")
    out_d = nc.dram_tensor("out", [DIM, S], BF16, kind="ExternalOutput")

    with tile.TileContext(nc) as tc:
        with tc.tile_pool(name="singles", bufs=1) as singles:
            ident = singles.tile([128, 128], BF16)
            make_identity(nc, ident)
            ones_col_f = singles.tile([128, 1], F32)
            nc.vector.memset(ones_col_f, 1.0)
            ones_col = singles.tile([128, 1], BF16)
            nc.vector.tensor_copy(ones_col, ones_col_f)
            ones_row_f = singles.tile([1, 128], F32)
            nc.vector.memset(ones_row_f, 1.0)
            ones_row_r = singles.tile([1, 128], F32R)
            nc.vector.tensor_copy(ones_row_r, ones_row_f)
            eps_t = singles.tile([1, 1], F32)
            nc.vector.memset(eps_t, EPS)
            qg_t = singles.tile([1, HQ], F32)
            nc.scalar.dma_start(out=qg_t, in_=qg_d[:, :])
            # per-m broadcast rows for the rmsnorm scale matmul: row m<4 is
            # q_gain[m] replicated, row 4 (k head) is ones.
            qg_rows = singles.tile([1, HQ + 1, 128], F32R)
            for m in range(HQ):
                nc.vector.tensor_scalar_mul(
                    qg_rows[0:1, m, :], ones_row_f, qg_t[0:1, m:m + 1])
            nc.vector.tensor_copy(qg_rows[0:1, HQ, :], ones_row_f)

            cosf = singles.tile([128, S], BF16)
            sinf = singles.tile([128, S], BF16)
            w_sb = singles.tile([128, 6, NKT, 128], BF16)
            wo_sb = singles.tile([128, HQ, NST, 128], BF16)

            # multiplicative causal masks for the 4 diagonal offsets,
            # replicated across a 2-head free dim: masks2[:, dlt, j, s] =
            # 1 if s - p - 128*dlt >= 0 else 0.
            masks2 = singles.tile([128, 4, 2, 512], BF16)
            for dlt in range(4):
                mk = masks2[:, dlt, :, :]
                nc.gpsimd.memset(mk, 1.0)
                nc.gpsimd.affine_select(
                    out=mk, in_=mk, compare_op=mybir.AluOpType.is_ge,
                    fill=0.0, base=-128 * dlt, pattern=[[0, 2], [1, 512]],
                    channel_multiplier=-1)

            # persistent activations
            qn = singles.tile([128, HQ + 1, S], BF16)   # normed pre-rope q/k
            qtr = singles.tile([128, HQ, S], BF16)      # roped q
            ktr = singles.tile([128, S], BF16)          # roped k
            vtb = singles.tile([128, S], BF16)          # v, feature-major
            v_nat = singles.tile([128, NST, D], BF16)   # v, [sk, t, d]
            yt = singles.tile([128, HQ, S], BF16)       # attn out

            # ---------------------------------------------------------------
            # attention pass emitter, shared by the phase-P interleave (w=1)
            # and phase A (w=2). Yields thunks; each emits one step's
            # instructions. s/py PSUM tiles come from (pool, tag) handles.
            def attn_pass_units(c, h0, w, spool, stag, sbufs, pypool, pytag,
                                apool, fill, skew=2):
                nt = 4 * c + 4
                sl = slice(512 * c, 512 * (c + 1))
                box = {"pend": [], "navs": 0}

                def emit_av(stop):
                    pp, pt = box["pend"].pop(0)
                    for j in range(w):
                        nc.tensor.matmul(
                            box["py"][:, j, :], v_nat[:, pt, :], pp[:, j, :],
                            start=(box["navs"] == 0), stop=stop)
                    box["navs"] += 1

                def step(i, t):
                    def f():
                        s2 = spool.tile([128, w, 512], F32, tag=stag,
                                        bufs=sbufs, name="s2")
                        for j in range(w):
                            nc.tensor.matmul(
                                s2[:, j, :], ktr[:, 128 * t:128 * (t + 1)],
                                qtr[:, h0 + j, sl], start=True, stop=True)
                        # AV lags `skew` steps behind the scores so PE never
                        # waits on the exp(+mask) chain.
                        if len(box["pend"]) >= skew:
                            emit_av(stop=False)
                        fill(i)
                        pt2 = apool.tile([128, w, 512], BF16, tag=f"pt{w}",
                                         bufs=skew + 2, name="pt")
                        nc.scalar.activation(
                            out=pt2, in_=s2,
                            func=mybir.ActivationFunctionType.Exp,
                            scale=SCALE)
                        if t >= 4 * c:
                            nc.vector.tensor_mul(
                                pt2, pt2, masks2[:, t - 4 * c, 0:w, :])
                        if i == 0:
                            box["py"] = pypool.tile([128, w, 512], F32,
                                                    tag=pytag, bufs=1,
                                                    name="py")
                            box["lacc"] = apool.tile([128, w, 512], BF16,
                                                     tag=f"lacc{w}", bufs=2,
                                                     name="lacc")
                            nc.vector.tensor_copy(box["lacc"], pt2)
                        else:
                            nc.vector.tensor_add(box["lacc"], box["lacc"],
                                                 pt2)
                        box["pend"].append((pt2, t))
                    return f

                def av_final():
                    while box["pend"]:
                        emit_av(stop=(len(box["pend"]) == 1))

                def pl_part():
                    pl2 = spool.tile([128, w, 512], F32, tag=stag, bufs=sbufs,
                                     name="pl2")
                    for j in range(w):
                        nc.tensor.matmul(pl2[0:1, j, :], ones_col,
                                         box["lacc"][:, j, :], start=True,
                                         stop=True)
                    linv = apool.tile([1, w, 512], F32R, tag=f"linv{w}",
                                      bufs=2, name="linv")
                    with nc.allow_low_precision(reason="f32r 1/l row"):
                        nc.vector.reciprocal(linv, pl2[0:1, :, :])
                    box["linv"] = linv

                def rb_part_a():
                    rb2 = spool.tile([128, w, 512], F32, tag=stag, bufs=sbufs,
                                     name="rb2")
                    for j in range(w):
                        nc.tensor.matmul(rb2[:, j, :], ones_row_r,
                                         box["linv"][0:1, j, :], start=True,
                                         stop=True)
                    # TensorTensor may read only one PSUM operand; stage the
                    # broadcast in SBUF.
                    rbs = apool.tile([128, w, 512], BF16, tag=f"rbs{w}",
                                     bufs=2, name="rbs")
                    nc.scalar.copy(rbs, rb2)
                    nc.vector.tensor_mul(yt[:, h0:h0 + w, sl], box["py"], rbs)

                # masked (diagonal) tiles first; pass-end tile has the short
                # score->exp->lacc chain.
                ts_order = list(range(nt - 1, -1, -1))
                for i, t in enumerate(ts_order):
                    yield step(i, t)
                yield av_final
                yield pl_part
                yield rb_part_a

            _loop_stack = contextlib.ExitStack()
            if loop > 1:
                _loop_stack.enter_context(tc.For_i(0, loop))

            # ------------- phase P: projections + rmsnorm + rope -------------
            with tc.tile_pool(name="xtp", bufs=2) as xtp, \
                 tc.tile_pool(name="qkp", bufs=1) as qkp, \
                 tc.tile_pool(name="rp", bufs=1) as rp, \
                 tc.tile_pool(name="pps", bufs=1, space="PSUM") as pps:

                xt_r = xt_d[:, :].rearrange("(kt p) s -> p kt s", p=128)
                w_r = wqkv_d[:, :].rearrange("p (m kt j) -> p m kt j",
                                             m=6, kt=NKT)
                xcs = []
                # fine-grained prologue ordered by first-use time: per-m
                # weight blocks interleaved with kt-group slices of chunk 0,
                # then rope tables and chunk 1. Chunk 2/3 x and wo issue
                # later, from inside the chunk loop, so the serialized DMA
                # stream matches consumption order.
                xc0 = xtp.tile([128, NKT, 512], BF16, tag="xt", name="xc")
                nc.sync.dma_start(out=w_sb[:, 0, 0:4, :],
                                  in_=w_r[:, 0, 0:4, :])
                nc.sync.dma_start(out=xc0[:, 0:2, :], in_=xt_r[:, 0:2, 0:512])
                nc.sync.dma_start(out=xc0[:, 2:4, :], in_=xt_r[:, 2:4, 0:512])
                nc.sync.dma_start(out=w_sb[:, 0, 4:16, :],
                                  in_=w_r[:, 0, 4:16, :])
                nc.sync.dma_start(out=xc0[:, 4:8, :], in_=xt_r[:, 4:8, 0:512])
                nc.sync.dma_start(out=w_sb[:, 1, :, :], in_=w_r[:, 1, :, :])
                nc.sync.dma_start(out=xc0[:, 8:12, :],
                                  in_=xt_r[:, 8:12, 0:512])
                nc.sync.dma_start(out=w_sb[:, 2, :, :], in_=w_r[:, 2, :, :])
                nc.sync.dma_start(out=xc0[:, 12:16, :],
                                  in_=xt_r[:, 12:16, 0:512])
                for m in (3, 4, 5):
                    nc.sync.dma_start(out=w_sb[:, m, :, :], in_=w_r[:, m, :, :])
                xcs.append(xc0)
                nc.sync.dma_start(out=cosf, in_=cosf_d[:, :])
                nc.sync.dma_start(out=sinf, in_=sinf_d[:, :])
                xc1 = xtp.tile([128, NKT, 512], BF16, tag="xt", name="xc")
                nc.sync.dma_start(out=xc1, in_=xt_r[:, :, 512:1024])
                xcs.append(xc1)

                def load_xc(c):
                    xc = xtp.tile([128, NKT, 512], BF16, tag="xt", name="xc")
                    nc.sync.dma_start(out=xc,
                                      in_=xt_r[:, :, 512 * c:512 * (c + 1)])
                    xcs.append(xc)

                saved = {}

                def proj_mms(c, m, extra=()):
                    """16 accumulating matmuls; thunks from `extra` are
                    emitted after kt 3/7/11/15 to interleave other PE work."""
                    extra = list(extra)
                    pp = pps.tile([128, 512], F32, tag="pp", bufs=2, name="pp")
                    for kt in range(NKT):
                        nc.tensor.matmul(
                            pp, w_sb[:, m, kt, :], xcs[c][:, kt, :],
                            start=(kt == 0), stop=(kt == NKT - 1))
                        if kt % 4 == 3 and extra:
                            extra.pop(0)()
                    for f in extra:
                        f()
                    if m == 5:
                        nc.scalar.copy(vtb[:, 512 * c:512 * (c + 1)], pp)
                    else:
                        qf = qkp.tile([128, 512], BF16, tag="qf", bufs=4,
                                      name="qf")
                        nc.scalar.copy(qf, pp)
                        sq = qkp.tile([128, 512], BF16, tag="sq", bufs=4,
                                      name="sq")
                        nc.vector.tensor_mul(sq, qf, qf)
                        saved[(c, m)] = (qf, sq)

                def ssq_part(c, m):
                    qf, sq = saved[(c, m)]
                    ssq = pps.tile([1, 512], F32, tag="ssq", bufs=1,
                                   name="ssq")
                    nc.tensor.matmul(ssq, ones_col, sq, start=True, stop=True)
                    row = rp.tile([1, 512], F32, tag="row", bufs=3, name="row")
                    nc.scalar.activation(
                        out=row, in_=ssq,
                        func=mybir.ActivationFunctionType.Sqrt,
                        bias=eps_t[:, :], scale=1.0 / D)
                    rowr = rp.tile([1, 512], F32R, tag="rowr", bufs=3,
                                   name="rowr")
                    with nc.allow_low_precision(reason="f32r 1/rms row"):
                        nc.vector.reciprocal(rowr, row)
                    saved[(c, m)] = (qf, rowr)

                def rb_part(c, m):
                    sl = slice(512 * c, 512 * (c + 1))
                    qf, rowr = saved.pop((c, m))
                    rb = pps.tile([128, 512], F32, tag="rb", bufs=1, name="rb")
                    nc.tensor.matmul(rb, qg_rows[0:1, m, :], rowr,
                                     start=True, stop=True)
                    nc.vector.tensor_mul(qn[:, m, sl], qf, rb)

                def rope_part(c, m):
                    # rope: dst = qn*cos + rot(qn)*sin, sin sign-baked.
                    # Runs one chunk behind the projection (rope only gates
                    # attention) so DVE never head-of-line blocks on it.
                    sl = slice(512 * c, 512 * (c + 1))
                    rot = rp.tile([128, 512], BF16, tag="rot", bufs=3,
                                  name="rot")
                    nc.sync.dma_start(out=rot[0:64, :], in_=qn[64:128, m, sl])
                    nc.sync.dma_start(out=rot[64:128, :], in_=qn[0:64, m, sl])
                    m1 = rp.tile([128, 512], BF16, tag="m1", bufs=2, name="m1")
                    nc.vector.tensor_mul(m1, qn[:, m, sl], cosf[:, sl])
                    m2 = rp.tile([128, 512], BF16, tag="m2", bufs=2, name="m2")
                    nc.vector.tensor_mul(m2, rot, sinf[:, sl])
                    rdst = ktr[:, sl] if m == HQ else qtr[:, m, sl]
                    nc.vector.tensor_add(rdst, m1, m2)

                def v_transpose(c):
                    vt = pps.tile([128, 4, 128], BF16, tag="vt", bufs=1,
                                  name="vt")
                    for k in range(4):
                        nc.tensor.transpose(
                            vt[:, k, :],
                            vtb[:, 512 * c + 128 * k:512 * c + 128 * (k + 1)],
                            ident)
                    nc.scalar.copy(v_nat[:, 4 * c:4 * (c + 1), :], vt)

                # chunk-0 attention: 4 one-head passes whose units are
                # interleaved into the proj matmul streams of chunks 2-3.
                a0 = {}
                for h in range(HQ):
                    a0[h] = attn_pass_units(
                        0, h, 1, pps, "as", 2, pps, "apy", rp,
                        lambda i: None)

                def _noop():
                    pass

                # (c, m) -> list of attn-0 units to interleave there; the
                # noops delay pl until the exp->mask->lacc chain of the last
                # step (~2.5us) has drained.
                a0_sched = {}
                for h, (cm_a, cm_b) in enumerate(
                        [((2, 1), (2, 2)), ((2, 3), (2, 4)),
                         ((2, 5), (3, 0)), ((3, 1), (3, 2))]):
                    a0_sched[cm_a] = list(itertools.islice(a0[h], 5))
                    a0_sched[cm_b] = [_noop, _noop] + list(a0[h])

                for c in range(NCH):
                    if 1 <= c < NCH - 1:
                        load_xc(c + 1)      # prefetch one chunk ahead
                    for m in range(6):
                        if c == 1 and m == 3:
                            nc.sync.dma_start(
                                out=wo_sb, in_=wot_d[:, :].rearrange(
                                    "p (h jt j) -> p h jt j", h=HQ, jt=NST))
                        proj_mms(c, m, a0_sched.get((c, m), ()))
                        if c > 0 and m == 0:
                            v_transpose(c - 1)
                        if c > 0 and m <= 4:
                            rope_part(c - 1, m)
                        if m >= 1:
                            ssq_part(c, m - 1)
                        if m >= 3:
                            rb_part(c, m - 3)
                    rb_part(c, 3)
                    rb_part(c, 4)
                for m in range(5):
                    rope_part(NCH - 1, m)
                v_transpose(NCH - 1)

            # ------------- phase A: attention chunks 1-3 + out-proj ---------
            with tc.tile_pool(name="att", bufs=1) as att, \
                 tc.tile_pool(name="ps_s", bufs=2, space="PSUM") as ps_s, \
                 tc.tile_pool(name="ps_y", bufs=1, space="PSUM") as ps_y, \
                 tc.tile_pool(name="ps_o", bufs=2, space="PSUM") as ps_o:

                # out-projection filler units: one PE matmul each (plus
                # trailing copy+DMA on the 4th). Copies round-robin over
                # Pool/ACT/DVE; store DMAs alternate the SP and ACT queues.
                rr = [0]

                def outproj_units(cp, tail=False):
                    slp = slice(512 * cp, 512 * (cp + 1))
                    for jt in range(NST):
                        po = ps_o.tile([128, 512], F32, tag="po", bufs=2,
                                       name="po")
                        for h in range(HQ):
                            def unit(po=po, jt=jt, h=h):
                                nc.tensor.matmul(
                                    po, wo_sb[:, h, jt, :], yt[:, h, slp],
                                    start=(h == 0), stop=(h == HQ - 1))
                                if h == HQ - 1:
                                    ot = att.tile([128, 512], BF16, tag="ot",
                                                  bufs=3, name="ot")
                                    r = rr[0] % 3
                                    rr[0] += 1
                                    if tail:
                                        # end-of-program drain: fastest copy
                                        # engines, stores on the idle SP queue
                                        if r % 2:
                                            nc.scalar.copy(ot, po)
                                        else:
                                            nc.vector.tensor_copy(ot, po)
                                        nc.sync.dma_start(
                                            out=out_d[128 * jt:128 * (jt + 1),
                                                      slp], in_=ot)
                                        return
                                    if r == 2:
                                        nc.vector.tensor_copy(ot, po)
                                    else:
                                        nc.scalar.copy(ot, po)
                                    (nc.sync if rr[0] % 2 else
                                     nc.scalar).dma_start(
                                        out=out_d[128 * jt:128 * (jt + 1),
                                                  slp],
                                        in_=ot)
                            yield unit

                pending = outproj_units(0)

                def emit_fillers(n):
                    for _ in range(n):
                        u = next(pending, None)
                        if u is None:
                            return
                        u()

                for c in range(1, NCH):
                    nt = 4 * c + 4
                    # 16 fillers per pass go to the pass-end bursts; the rest
                    # of the previous chunk's 64 spread across the t-steps.
                    quota = 32.0 / (2 * nt)
                    owed = [0.0]

                    def spread(i, quota=quota, owed=owed):
                        if i == 0:
                            return
                        owed[0] += quota
                        emit_fillers(int(owed[0]))
                        owed[0] -= int(owed[0])

                    for hp in range(2):
                        units = list(attn_pass_units(
                            c, 2 * hp, 2, ps_s, "s", 2, ps_y, "py", att,
                            spread))
                        for u in units[:-2]:    # steps + av_final
                            u()
                        emit_fillers(4)
                        units[-2]()             # pl + recip
                        emit_fillers(8)
                        units[-1]()             # rb + yt-mul
                        emit_fillers(2)
                    # chain this chunk's out-proj behind any leftovers so the
                    # chunk boundary (yt of c not yet normalized) still has
                    # fillers to run.
                    pending = itertools.chain(
                        pending, outproj_units(c, tail=(c == NCH - 1)))

                emit_fillers(1 << 30)           # drain everything left

            _loop_stack.close()

    nc.compile()
    return nc


def _rope_tables():
    inv_freq = 1.0 / (10000.0 ** (np.arange(0, D, 2, dtype=np.float32) / D))
    t = np.arange(S, dtype=np.float32)
    freqs = np.outer(t, inv_freq)          # [S, 64] f32
    cos = np.cos(freqs).T                  # [64, S]
    sin = np.sin(freqs).T
    bf = ml_dtypes.bfloat16
    cosf = np.ascontiguousarray(np.concatenate([cos, cos], 0)).astype(bf)
    sinf = np.ascontiguousarray(np.concatenate([sin, -sin], 0)).astype(bf)
    return cosf, sinf


def _make_in_maps(x, wq, wk, wv, wo, q_gain):
    bf = ml_dtypes.bfloat16
    cosf, sinf = _rope_tables()
    in_maps = []
    for core in range(8):
        b, g = divmod(core, G)
        xt = np.ascontiguousarray(x[b].T).astype(bf)
        wfull = np.concatenate(
            [wq[MQ * g:MQ * (g + 1)],
             wk[D * g:D * (g + 1)],
             wv[D * g:D * (g + 1)]], axis=0)             # [KM, DIM]
        # -> [p, m, kt, mj]: per-m contiguous blocks for the per-m DMAs
        wqkv = np.ascontiguousarray(
            wfull.reshape(6, 128, NKT, 128).transpose(3, 0, 2, 1).reshape(128, -1)
        ).astype(bf)
        wot = np.ascontiguousarray(wo[:, MQ * g:MQ * (g + 1)].T)  # [MQ, DIM]
        wot = np.ascontiguousarray(
            wot.reshape(HQ, 128, NST, 128).transpose(1, 0, 2, 3).reshape(128, -1)
        ).astype(bf)
        qg = np.ascontiguousarray(q_gain[HQ * g:HQ * (g + 1)].reshape(1, HQ))
        in_maps.append({
            "xt": xt, "wqkv": wqkv, "wot": wot,
            "cosf": cosf, "sinf": sinf, "qg": qg.astype(np.float32),
        })
    return in_maps


def kernel(x, wq, wk, wv, wo, q_gain):
    x = np.asarray(x, dtype=np.float32)
    wq = np.asarray(wq, dtype=np.float32)
    wk = np.asarray(wk, dtype=np.float32)
    wv = np.asarray(wv, dtype=np.float32)
    wo = np.asarray(wo, dtype=np.float32)
    q_gain = np.asarray(q_gain, dtype=np.float32)

    if "nc" not in _CACHED:
        _CACHED["nc"] = _build_program()
    nc = _CACHED["nc"]

    in_maps = _make_in_maps(x, wq, wk, wv, wo, q_gain)
    res = run_bass_kernel_spmd(nc, in_maps, core_ids=list(range(8)))
    outs = res.results

    y = np.empty((B, S, DIM), dtype=np.float32)
    for b in range(B):
        acc = np.zeros((DIM, S), dtype=np.float32)
        for g in range(G):
            acc += outs[G * b + g]["out"].astype(np.float32)
        y[b] = acc.T
    return y
